# revision 1
# baseline (speedup 1.0000x reference)
"""DDiT block (adaLN attention + MLP) on 8 Trainium2 NeuronCores.

Sharding: cores 0-3 -> batch 0, cores 4-7 -> batch 1. Within a 4-core
batch group: attention is sharded by heads (4 heads/core, full sequence);
after the attention out-projection a grouped ReduceScatter sums the
per-head partial outputs and hands each core a 512-token slice, on which
it runs the (token-sharded) MLP.

Host prep folds the adaLN modulation into weights/biases:
  - ada = c @ ada_w.T + ada_b is computed on host (12 MFLOP)
  - LN scale A = norm_w * (1 + sc); the shift's contribution to each
    linear layer is folded into that layer's bias (B @ W.T)
  - gates g_msa / g_mlp are folded into w_out / mlp_w2 rows
All weights are shipped pre-transposed ([d_in, d_out]) in bf16, so the
device only ever runs natural lhsT.T @ rhs matmuls.

Device pipeline per core: token-major LN1 (bn_stats) -> PE-transpose ->
q,k feature-major + v token-major projections -> per head: scoresT =
K@Q.T (2-head packed via tile_position), exp on ScalarE (no max
subtraction; scores are bounded), attn@V with a ones-augmented V giving
the softmax denominator for free, delayed division -> out-projection ->
ReduceScatter -> residual + LN2 -> MLP (gelu bias-folded) -> residual.
"""

import numpy as np

import concourse.bass as bass
import concourse.mybir as mybir
import concourse.tile as tile
from concourse import bacc
from concourse.bass_utils import run_bass_kernel_spmd
from concourse.masks import make_identity

B, S, D, H, HD = 2, 2048, 1024, 16, 64
DFF = 4 * D
TOK = S // 4          # tokens per core for the MLP phase
EPS = 1e-5
GROUPS = [[0, 1, 2, 3], [4, 5, 6, 7]]
F32 = mybir.dt.float32
BF16 = mybir.dt.bfloat16
AF = mybir.ActivationFunctionType
ALU = mybir.AluOpType

_CACHE = {}


# ---------------------------------------------------------------- host prep

def _f(v):
    return np.ascontiguousarray(np.asarray(v, dtype=np.float32))


def _bf(a):
    import ml_dtypes
    return np.ascontiguousarray(a.astype(ml_dtypes.bfloat16))


def host_prep(inp):
    x, c = _f(inp["x"]), _f(inp["c"])
    norm1_w, norm2_w = _f(inp["norm1_w"]), _f(inp["norm2_w"])
    w_qkv, w_out = _f(inp["w_qkv"]), _f(inp["w_out"])
    mlp_w1, mlp_b1 = _f(inp["mlp_w1"]), _f(inp["mlp_b1"])
    mlp_w2, mlp_b2 = _f(inp["mlp_w2"]), _f(inp["mlp_b2"])
    ada_w, ada_b = _f(inp["ada_w"]), _f(inp["ada_b"])

    ada = c @ ada_w.T + ada_b                      # [B, 6D]
    sh_msa, sc_msa, g_msa, sh_mlp, sc_mlp, g_mlp = np.split(ada, 6, axis=1)
    A1 = norm1_w[None] * (1.0 + sc_msa)            # [B, D]
    A2 = norm2_w[None] * (1.0 + sc_mlp)
    bias_qkv = sh_msa @ w_qkv.T                    # [B, 3D]
    bias1 = mlp_b1[None] + sh_mlp @ mlp_w1.T       # [B, DFF]
    bias2 = g_mlp * mlp_b2[None]                   # [B, D]

    wq, wk, wv = w_qkv[0:D], w_qkv[D:2 * D], w_qkv[2 * D:3 * D]
    w1T = _bf(mlp_w1.T.copy())                     # [D, DFF]
    # rt-major blocks [32, D, 128] so each rt's 8 lhsT chunks DMA contiguously
    w1blk = np.ascontiguousarray(
        w1T.reshape(D, 32, 128).transpose(1, 0, 2))  # [32, D, 128] bf16

    in_maps = []
    for cid in range(8):
        b, r = cid // 4, cid % 4
        hsl = slice(256 * r, 256 * r + 256)
        woutg = g_msa[b][:, None] * w_out          # [D, D]
        w2g = g_mlp[b][:, None] * mlp_w2           # [D, DFF]
        in_maps.append({
            "x_b": _bf(x[b]),
            "x_res": np.ascontiguousarray(np.concatenate(
                [x[b][512 * t2 + 128 * r:512 * t2 + 128 * r + 128]
                 for t2 in range(4)])),
            "a1": np.ascontiguousarray(A1[b]),
            "a2": np.ascontiguousarray(A2[b]),
            "wqkT": _bf(np.vstack([wq[hsl], wk[hsl]]).T.copy()),   # [D, 512]
            "bias_qk": np.ascontiguousarray(np.concatenate(
                [bias_qkv[b, hsl],
                 bias_qkv[b, D + 256 * r:D + 256 * r + 256]])),    # [512]
            "wvT": _bf(wv[hsl].T.copy()),                          # [D, 256]
            "bias_v": np.ascontiguousarray(
                bias_qkv[b, 2 * D + 256 * r:2 * D + 256 * r + 256]),
            "woutT": _bf(woutg[:, hsl].T.copy()),                  # [256, D]
            "w1blk": w1blk,                                        # [32, D, 128]
            "bias1": np.ascontiguousarray(bias1[b]),
            "w2gT": _bf(w2g.T.copy()),                             # [DFF, D]
            "bias2": np.ascontiguousarray(bias2[b]),
        })
    return in_maps


# ------------------------------------------------------------- device build

def _bc(ap, p=128):
    """Broadcast a DRAM row AP across p partitions (stride-0 partition dim)."""
    return bass.AP(tensor=ap.tensor, offset=ap.offset,
                   ap=[[0, p]] + [list(d) for d in ap.ap])


def build_program(reps=1):
    nc = bacc.Bacc("TRN2", target_bir_lowering=False, debug=False, num_devices=8)

    x_d = nc.dram_tensor("x_b", [S, D], BF16, kind="ExternalInput")
    xr_d = nc.dram_tensor("x_res", [TOK, D], F32, kind="ExternalInput")
    a1_d = nc.dram_tensor("a1", [D], F32, kind="ExternalInput")
    a2_d = nc.dram_tensor("a2", [D], F32, kind="ExternalInput")
    wqk_d = nc.dram_tensor("wqkT", [D, 512], BF16, kind="ExternalInput")
    bqk_d = nc.dram_tensor("bias_qk", [512], F32, kind="ExternalInput")
    wv_d = nc.dram_tensor("wvT", [D, 256], BF16, kind="ExternalInput")
    bv_d = nc.dram_tensor("bias_v", [256], F32, kind="ExternalInput")
    wo_d = nc.dram_tensor("woutT", [256, D], BF16, kind="ExternalInput")
    w1_d = nc.dram_tensor("w1blk", [32, D, 128], BF16, kind="ExternalInput")
    b1_d = nc.dram_tensor("bias1", [DFF], F32, kind="ExternalInput")
    w2_d = nc.dram_tensor("w2gT", [DFF, D], BF16, kind="ExternalInput")
    b2_d = nc.dram_tensor("bias2", [D], F32, kind="ExternalInput")
    out_d = nc.dram_tensor("out", [TOK, D], F32, kind="ExternalOutput")

    with tile.TileContext(nc, num_cores=8) as tc:
        for _ in range(reps):
            _body(nc, tc, x_d, xr_d, a1_d, a2_d, wqk_d, bqk_d, wv_d, bv_d,
                  wo_d, w1_d, b1_d, w2_d, b2_d, out_d)
    nc.compile()
    return nc


def _body(nc, tc, x_d, xr_d, a1_d, a2_d, wqk_d, bqk_d, wv_d, bv_d,
          wo_d, w1_d, b1_d, w2_d, b2_d, out_d):
    mm = nc.tensor.matmul

    from contextlib import ExitStack
    with ExitStack() as outer:
        consts = outer.enter_context(tc.tile_pool(name="consts", bufs=1))
        mlpre = outer.enter_context(tc.tile_pool(name="mlpre", bufs=1))
        x2 = [mlpre.tile([128, D], F32, tag=f"x2_{t}", name=f"x2_{t}")
              for t in range(4)]
        h2T = [mlpre.tile([128, TOK], BF16, tag=f"h2T{dc}", name=f"h2T{dc}")
               for dc in range(8)]
        dram = outer.enter_context(tc.tile_pool(name="dram", bufs=1, space="DRAM"))

        # ---- constants
        ident = consts.tile([128, 128], BF16, tag="ident", name="ident")
        make_identity(nc, ident)
        eps_t = consts.tile([128, 1], F32, tag="eps", name="eps")
        nc.vector.memset(eps_t, EPS)
        ones_r = consts.tile([1, 64], BF16, tag="ones_r", name="ones_r")
        nc.vector.memset(ones_r, 1.0)
        a1bc = consts.tile([128, D], F32, tag="a1bc", name="a1bc")
        nc.sync.dma_start(out=a1bc, in_=_bc(a1_d[:]))
        a2bc = consts.tile([128, D], F32, tag="a2bc", name="a2bc")
        nc.sync.dma_start(out=a2bc, in_=_bc(a2_d[:]))
        b2bc = consts.tile([128, D], F32, tag="b2bc", name="b2bc")
        nc.sync.dma_start(out=b2bc, in_=_bc(b2_d[:]))
        bvbc = consts.tile([128, 256], F32, tag="bvbc", name="bvbc")
        nc.sync.dma_start(out=bvbc, in_=_bc(bv_d[:]))
        bqk_t = consts.tile([128, 4], F32, tag="bqk", name="bqk")
        nc.sync.dma_start(out=bqk_t, in_=bass.AP(
            tensor=bqk_d[:].tensor, offset=0, ap=[[1, 128], [128, 4]]))
        b1_t = consts.tile([128, 32], F32, tag="b1t", name="b1t")
        nc.sync.dma_start(out=b1_t, in_=bass.AP(
            tensor=b1_d[:].tensor, offset=0, ap=[[1, 128], [128, 32]]))

        # ---- DRAM scratch for the chunked collective (one tile per q-block)
        y_part = [dram.tile([512, D], BF16, tag=f"y_part{i}", name=f"y_part{i}")
                  for i in range(4)]
        y_sum = [dram.tile([128, D], BF16, tag=f"y_sum{i}", name=f"y_sum{i}")
                 for i in range(4)]


        with ExitStack() as attctx:
            wpool = attctx.enter_context(tc.tile_pool(name="wpool", bufs=1))
            acts = attctx.enter_context(tc.tile_pool(name="acts", bufs=1))

            wqk_sb = [wpool.tile([128, 512], BF16, tag=f"wqk{k}", name=f"wqk{k}") for k in range(8)]
            wv_sb = [wpool.tile([128, 256], BF16, tag=f"wv{k}", name=f"wv{k}") for k in range(8)]
            wo_sb = [wpool.tile([128, D], BF16, tag=f"wo{k}", name=f"wo{k}") for k in range(2)]

            qkT = [acts.tile([128, S], BF16, tag=f"qkT{rt}", name=f"qkT{rt}") for rt in range(4)]
            v_aug = [acts.tile([128, 4, 65], BF16, tag=f"vaug{tt}", name=f"vaug{tt}") for tt in range(16)]
            attnT = [acts.tile([128, S], BF16, tag=f"attnT{i}", name=f"attnT{i}") for i in range(2)]

            # ================= P1: LN1 + modulate + transpose =================
            with tc.tile_pool(name="hTp", bufs=1) as hTp:
                hT = [hTp.tile([128, S], BF16, tag=f"hT{dc}", name=f"hT{dc}") for dc in range(8)]
                with tc.tile_pool(name="lnp", bufs=2) as lnp, \
                     tc.tile_pool(name="psT", bufs=4, space="PSUM") as psT:
                    for tt in range(16):
                        xt = lnp.tile([128, D], BF16, tag="xt", name="xt")
                        nc.sync.dma_start(out=xt, in_=x_d[tt * 128:(tt + 1) * 128, :])
                        st = lnp.tile([128, 2, 6], F32, tag="st", name="st")
                        xg = xt.rearrange("p (g d) -> p g d", g=2)
                        for g in range(2):
                            nc.vector.bn_stats(out=st[:, g, :], in_=xg[:, g, :])
                        mv = lnp.tile([128, 2], F32, tag="mv", name="mv")
                        nc.vector.bn_aggr(out=mv, in_=st)
                        rstd = lnp.tile([128, 1], F32, tag="rstd", name="rstd")
                        nc.scalar.activation(out=rstd, in_=mv[:, 1:2],
                                             func=AF.Sqrt, bias=eps_t, scale=1.0)
                        nc.vector.reciprocal(out=rstd, in_=rstd)
                        xh = lnp.tile([128, D], F32, tag="xh", name="xh")
                        nc.vector.tensor_scalar(out=xh, in0=xt, scalar1=mv[:, 0:1],
                                                scalar2=rstd, op0=ALU.subtract,
                                                op1=ALU.mult)
                        ht = lnp.tile([128, D], BF16, tag="ht", name="ht")
                        nc.gpsimd.tensor_tensor(out=ht, in0=xh, in1=a1bc, op=ALU.mult)
                        for dc in range(8):
                            pt = psT.tile([128, 128], BF16, tag="pt", name="pt")
                            nc.tensor.transpose(pt, ht[:, dc * 128:(dc + 1) * 128], ident)
                            dst = hT[dc][:, tt * 128:(tt + 1) * 128]
                            if dc % 2 == 0:
                                nc.scalar.copy(out=dst, in_=pt)
                            else:
                                nc.vector.tensor_copy(out=dst, in_=pt)

                for kc in range(8):
                    nc.sync.dma_start(out=wqk_sb[kc], in_=wqk_d[kc * 128:(kc + 1) * 128, :])
                    nc.sync.dma_start(out=wv_sb[kc], in_=wv_d[kc * 128:(kc + 1) * 128, :])
                for kc in range(2):
                    nc.sync.dma_start(out=wo_sb[kc], in_=wo_d[kc * 128:(kc + 1) * 128, :])
                # ================= P2: q,k projection (feature-major) =========
                with tc.tile_pool(name="psQK", bufs=4, space="PSUM") as psQK, \
                     tc.tile_pool(name="psV", bufs=2, space="PSUM") as psV:
                    for rt in range(4):
                        for tb in range(4):
                            pm = psQK.tile([128, 512], F32, tag="pm", name="pm")
                            for kc in range(8):
                                mm(pm, lhsT=wqk_sb[kc][:, rt * 128:(rt + 1) * 128],
                                   rhs=hT[kc][:, tb * 512:(tb + 1) * 512],
                                   start=(kc == 0), stop=(kc == 7))
                            nc.vector.tensor_scalar_add(
                                out=qkT[rt][:, tb * 512:(tb + 1) * 512],
                                in0=pm, scalar1=bqk_t[:, rt:rt + 1])

                    # ============= P3: v projection (token-major) ==============
                    for tt in range(16):
                        pv = psV.tile([128, 256], F32, tag="pmv", name="pmv")
                        for kc in range(8):
                            mm(pv, lhsT=hT[kc][:, tt * 128:(tt + 1) * 128],
                               rhs=wv_sb[kc], start=(kc == 0), stop=(kc == 7))
                        nc.vector.memset(v_aug[tt], 1.0)
                        nc.vector.tensor_tensor(
                            out=v_aug[tt][:, :, 0:64],
                            in0=pv.rearrange("p (h d) -> p h d", h=4),
                            in1=bvbc.rearrange("p (h d) -> p h d", h=4),
                            op=ALU.add)

            # ================= P4/P5: attention + out-projection ==============
            with tc.tile_pool(name="attp", bufs=2) as attp, \
                 tc.tile_pool(name="att2", bufs=2) as att2, \
                 tc.tile_pool(name="mtmp", bufs=1) as mtmp, \
                 tc.tile_pool(name="psS", bufs=2, space="PSUM") as psS, \
                 tc.tile_pool(name="psN", bufs=1, space="PSUM") as psN, \
                 tc.tile_pool(name="psB", bufs=1, space="PSUM") as psB, \
                 tc.tile_pool(name="psT2", bufs=1, space="PSUM") as psT2, \
                 tc.tile_pool(name="psO", bufs=1, space="PSUM") as psO:
                def ln2_chunk(t2):
                    ys = mtmp.tile([128, D], BF16, tag="ys", name="ys")
                    nc.gpsimd.dma_start(out=ys, in_=y_sum[t2][:])
                    xr = mtmp.tile([128, D], F32, tag="xr", name="xr")
                    nc.gpsimd.dma_start(
                        out=xr, in_=xr_d[t2 * 128:(t2 + 1) * 128, :])
                    nc.vector.tensor_tensor(out=x2[t2], in0=xr, in1=ys,
                                            op=ALU.add)
                    st2 = mtmp.tile([128, 2, 6], F32, tag="st2", name="st2")
                    xg2 = x2[t2].rearrange("p (g d) -> p g d", g=2)
                    for g in range(2):
                        nc.vector.bn_stats(out=st2[:, g, :], in_=xg2[:, g, :])
                    mv2 = mtmp.tile([128, 2], F32, tag="mv2", name="mv2")
                    nc.vector.bn_aggr(out=mv2, in_=st2)
                    rstd2 = mtmp.tile([128, 1], F32, tag="rstd2", name="rstd2")
                    nc.scalar.activation(out=rstd2, in_=mv2[:, 1:2], func=AF.Sqrt,
                                         bias=eps_t, scale=1.0)
                    nc.vector.reciprocal(out=rstd2, in_=rstd2)
                    xh2 = mtmp.tile([128, D], F32, tag="xh2", name="xh2")
                    nc.vector.tensor_scalar(out=xh2, in0=x2[t2],
                                            scalar1=mv2[:, 0:1],
                                            scalar2=rstd2, op0=ALU.subtract,
                                            op1=ALU.mult)
                    h2 = mtmp.tile([128, D], BF16, tag="h2", name="h2")
                    nc.gpsimd.tensor_tensor(out=h2, in0=xh2, in1=a2bc,
                                            op=ALU.mult)
                    for dc in range(8):
                        pt2 = psT2.tile([128, 128], BF16, tag="pt2", name="pt2")
                        nc.tensor.transpose(
                            pt2, h2[:, dc * 128:(dc + 1) * 128], ident)
                        dst = h2T[dc][:, t2 * 128:(t2 + 1) * 128]
                        if dc % 2 == 0:
                            nc.scalar.copy(out=dst, in_=pt2)
                        else:
                            nc.vector.tensor_copy(out=dst, in_=pt2)


                for qb in range(4):
                    qsl = slice(qb * 512, (qb + 1) * 512)
                    exp_t = {}
                    for pair in range(2):
                        for kt in range(16):
                            for sub in range(2):
                                h = 2 * pair + sub
                                _ = h
                                psl = slice(sub * 64, (sub + 1) * 64)
                                ps = psS.tile([128, 512], F32, tag=f"scr{sub}", name=f"scr{sub}")
                                mm(ps,
                                   lhsT=qkT[2 + pair][psl, kt * 128:(kt + 1) * 128],
                                   rhs=qkT[pair][psl, qsl],
                                   start=True, stop=True,
                                   tile_position=(sub * 64, 0))
                                if kt % 3 == 2:
                                    # Schraudolph bf16 exp on DVE (ACT offload)
                                    ei = attp.tile([128, 512], mybir.dt.int16,
                                                   tag=f"e{sub}_{kt}", name=f"e{sub}_{kt}")
                                    nc.vector.tensor_scalar(
                                        out=ei, in0=ps, scalar1=23.083128,
                                        scalar2=16250.5, op0=ALU.mult, op1=ALU.add)
                                    exp_t[(h, kt)] = ei[:].bitcast(BF16)
                                else:
                                    ex = attp.tile([128, 512], BF16, tag=f"e{sub}_{kt}", name=f"e{sub}_{kt}")
                                    nc.scalar.activation(out=ex, in_=ps, func=AF.Exp,
                                                         scale=0.125)
                                    exp_t[(h, kt)] = ex
                    for h in range(4):
                        if True:
                            pn = psN.tile([128, 512], F32, tag="num", name="num")
                            for kc in range(16):
                                mm(pn[0:65, :], lhsT=v_aug[kc][:, h, :],
                                   rhs=exp_t[(h, kc)],
                                   start=(kc == 0), stop=(kc == 15))
                            rc = att2.tile([1, 512], BF16, tag="rc", name="rc")
                            with nc.allow_low_precision(reason="bf16 softmax denom"):
                                nc.vector.reciprocal(out=rc, in_=pn[64:65, :])
                            pb = psB.tile([64, 512], F32, tag="pb", name="pb")
                            mm(pb, lhsT=ones_r, rhs=rc, start=True, stop=True)
                            rcb = att2.tile([64, 512], BF16, tag="rcb", name="rcb")
                            nc.vector.tensor_copy(out=rcb, in_=pb)
                            if h % 2 == 0:
                                nc.vector.tensor_tensor(
                                    out=attnT[h // 2][0:64, qsl],
                                    in0=pn[0:64, :], in1=rcb, op=ALU.mult)
                            else:
                                ad = att2.tile([64, 512], BF16, tag="adiv", name="adiv")
                                nc.vector.tensor_tensor(
                                    out=ad, in0=pn[0:64, :], in1=rcb, op=ALU.mult)
                                nc.sync.dma_start(
                                    out=attnT[h // 2][64:128, qsl], in_=ad)
                    # out-projection for this q-block (token-major partial y)
                    for tt in range(4):
                        tok = qb * 512 + tt * 128
                        yb = att2.tile([128, D], BF16, tag="ysb", name="ysb")
                        for n in range(2):
                            po = psO.tile([128, 512], F32, tag="po", name="po")
                            for kc in range(2):
                                mm(po, lhsT=attnT[kc][:, tok:tok + 128],
                                   rhs=wo_sb[kc][:, n * 512:(n + 1) * 512],
                                   start=(kc == 0), stop=(kc == 1))
                            if n == 0:
                                nc.vector.tensor_copy(out=yb[:, 0:512], in_=po)
                            else:
                                nc.scalar.copy(out=yb[:, 512:1024], in_=po)
                        nc.sync.dma_start(
                            out=y_part[qb][tt * 128:(tt + 1) * 128, :], in_=yb)
                    nc.gpsimd.collective_compute(
                        "ReduceScatter", ALU.add, replica_groups=GROUPS,
                        ins=[y_part[qb].opt()], outs=[y_sum[qb].opt()])
                    if qb >= 1:
                        ln2_chunk(qb - 1)
                ln2_chunk(3)
        with tc.tile_pool(name="mlpp", bufs=1) as mlpp, \
             tc.tile_pool(name="w2pool", bufs=1) as w2pool, \
             tc.tile_pool(name="mstream", bufs=3) as mstream, \
             tc.tile_pool(name="mtmp", bufs=2) as mtmp:
            w2_sb = [w2pool.tile([128, D], BF16, tag=f"w2_{kc}", name=f"w2_{kc}")
                     for kc in range(32)]
            g1T = [mlpp.tile([128, TOK], BF16, tag=f"g1T{rt}", name=f"g1T{rt}") for rt in range(32)]

            # P8: mlp_w1 + gelu (feature-major out)
            with tc.tile_pool(name="psM1", bufs=4, space="PSUM") as psM1, \
                 tc.tile_pool(name="psM2", bufs=2, space="PSUM") as psM2:
                for rt in range(32):
                    nc.sync.dma_start(out=w2_sb[rt],
                                      in_=w2_d[rt * 128:(rt + 1) * 128, :])
                    w1t = mstream.tile([128, 8, 128], BF16, tag="w1rt", name="w1rt")
                    nc.sync.dma_start(
                        out=w1t,
                        in_=w1_d[rt].rearrange("(kc p) r -> p kc r", p=128))
                    pm1 = psM1.tile([128, 512], F32, tag="pm1", name="pm1")
                    for kc in range(8):
                        mm(pm1, lhsT=w1t[:, kc, :], rhs=h2T[kc],
                           start=(kc == 0), stop=(kc == 7))
                    nc.scalar.activation(out=g1T[rt], in_=pm1,
                                         func=AF.Gelu_apprx_tanh,
                                         bias=b1_t[:, rt:rt + 1], scale=1.0)

                # P9: mlp_w2 + bias2 + residual (token-major out)
                for t2 in range(4):
                    ob = mtmp.tile([128, D], F32, tag="ob", name="ob")
                    for n in range(2):
                        nsl = slice(n * 512, (n + 1) * 512)
                        pm2 = psM2.tile([128, 512], F32, tag="pm2", name="pm2")
                        for kc in range(32):
                            mm(pm2, lhsT=g1T[kc][:, t2 * 128:(t2 + 1) * 128],
                               rhs=w2_sb[kc][:, nsl],
                               start=(kc == 0), stop=(kc == 31))
                        tb = mtmp.tile([128, 512], F32, tag="tb9", name="tb9")
                        nc.vector.tensor_tensor(out=tb, in0=pm2, in1=b2bc[:, nsl],
                                                op=ALU.add)
                        nc.vector.tensor_tensor(out=ob[:, nsl], in0=tb,
                                                in1=x2[t2][:, nsl], op=ALU.add)
                    nc.sync.dma_start(out=out_d[t2 * 128:(t2 + 1) * 128, :], in_=ob)


# ----------------------------------------------------------------- kernel()

def _get_nc():
    if "nc" not in _CACHE:
        _CACHE["nc"] = build_program()
    return _CACHE["nc"]


def kernel(**inputs) -> np.ndarray:
    in_maps = host_prep(inputs)
    nc = _get_nc()
    res = run_bass_kernel_spmd(nc, in_maps, list(range(8)))
    out = np.zeros((B, S, D), np.float32)
    for cid in range(8):
        b, r = cid // 4, cid % 4
        o = res.results[cid]["out"]
        for t2 in range(4):
            out[b, 512 * t2 + 128 * r:512 * t2 + 128 * r + 128] = \
                o[128 * t2:128 * t2 + 128]
    return out

